# revision 1
# baseline (speedup 1.0000x reference)
"""Trainium2 Bass kernel for the shifted-slice-copy stereo cost volume.

Reference semantics (B=2, C=32, H=128, W=240, D=max_disp//4=48):
    out[:, :C,  d, :, w] = left[:, :, :, w]      if w >= d else 0
    out[:, C:,  d, :, w] = right[:, :, :, w - d] if w >= d else 0
    out shape [B, 2C, D, H, W] float32  (~755 MB)

Pure data movement (memory-regime): each core loads its input shard into
SBUF once, then streams one strided DMA store per disparity per half,
covering only the valid w >= d suffix of each row. The invalid (w < d)
prefix is never written: run_bass_kernel_spmd pre-zeros / donates
zero-filled ExternalOutput buffers, so the masked region is already zero.

Sharding: 8 cores = 2 batches x 4 channel-blocks of 8 channels. Every
core runs the identical program over all 48 disparities for its 8
channels of both halves, so the SPMD program is uniform across cores.

The two HWDGE rings (SP via nc.sync, ACT via nc.scalar) each carry one
half's stream; the 16 SDMA engines are saturated at ~47ns per ~900B
descriptor, which is the measured bottleneck (~298 GB/s/core aggregate,
~287us). Full-row variants with 7680B descriptors were tried and lost:
the compiler routes those stores to only 8 of 16 SDMA engines.
"""

import sys

import numpy as np

for _p in ("/opt/trn_rl_repo",):
    if _p not in sys.path:
        sys.path.insert(0, _p)

import concourse.bass as bass
from concourse import mybir
from concourse.bass_utils import run_bass_kernel_spmd

B, C, H, W = 2, 32, 128, 240
D = 48          # max_disp // 4
CPC = 8         # channels per core (C / 4 channel-blocks)
NCORES = 8

_NC_CACHE = None


def _build_bass():
    """One core's program: [CPC,H,W] left/right shard -> [2*CPC,D,H,W] out."""
    nc = bass.Bass()
    f32 = mybir.dt.float32
    left_c = nc.declare_dram_parameter("left_c", [CPC, H, W], f32, isOutput=False)
    right_c = nc.declare_dram_parameter("right_c", [CPC, H, W], f32, isOutput=False)
    out_c = nc.declare_dram_parameter("out_c", [2 * CPC, D, H, W], f32, isOutput=True)

    with (
        nc.sbuf_tensor("lsb", [H, CPC * W], f32) as lsb,
        nc.sbuf_tensor("rsb", [H, CPC * W], f32) as rsb,
        nc.semaphore("l_sem") as l_sem,
        nc.semaphore("r_sem") as r_sem,
        nc.Block() as block,
    ):
        lv = lsb[:, :].rearrange("p (c w) -> p c w", c=CPC)
        rv = rsb[:, :].rearrange("p (c w) -> p c w", c=CPC)

        # Two independent streams: SP engine (HWDGE) handles the left half,
        # ACT engine (HWDGE) the right half. Each: load shard into SBUF with
        # h on partitions ([h][c][w]), then one strided store per disparity
        # covering only the valid w >= d region (output is pre-zeroed).

        @block.sync
        def _(sync):
            sync.dma_start(
                lv, left_c[:, :, :].rearrange("c h w -> h c w")
            ).then_inc(l_sem, 16)
            sync.wait_ge(l_sem, 16)
            for d in range(D):
                # left half: out[c, d, h, w>=d] = left[c, h, w]
                sync.dma_start(
                    out_c[0:CPC, d, :, d:W].rearrange("c h w -> h c w"),
                    lv[:, :, d:W],
                ).then_inc(l_sem, 16)
            sync.wait_ge(l_sem, 16 * (D + 1))

        @block.scalar
        def _(scalar):
            scalar.dma_start(
                rv, right_c[:, :, :].rearrange("c h w -> h c w")
            ).then_inc(r_sem, 16)
            scalar.wait_ge(r_sem, 16)
            for d in range(D):
                # right half: out[CPC+c, d, h, w>=d] = right[c, h, w-d]
                scalar.dma_start(
                    out_c[CPC : 2 * CPC, d, :, d:W].rearrange("c h w -> h c w"),
                    rv[:, :, 0 : W - d],
                ).then_inc(r_sem, 16)
            scalar.wait_ge(r_sem, 16 * (D + 1))

    return nc


def _get_nc():
    global _NC_CACHE
    if _NC_CACHE is None:
        _NC_CACHE = _build_bass()
    return _NC_CACHE


def _shard_inputs(left, right):
    in_maps = []
    for i in range(NCORES):
        b, blk = divmod(i, 4)
        c0 = blk * CPC
        in_maps.append(
            {
                "left_c": np.ascontiguousarray(left[b, c0 : c0 + CPC]),
                "right_c": np.ascontiguousarray(right[b, c0 : c0 + CPC]),
            }
        )
    return in_maps


def _gather_outputs(results):
    out = np.empty((B, 2 * C, D, H, W), np.float32)
    for i in range(NCORES):
        b, blk = divmod(i, 4)
        c0 = blk * CPC
        oc = results[i]["out_c"]
        out[b, c0 : c0 + CPC] = oc[:CPC]
        out[b, C + c0 : C + c0 + CPC] = oc[CPC:]
    return out


def run_sharded(left, right, **run_kwargs):
    """Compile+run the SPMD kernel; returns (full_output, BassKernelResults)."""
    res = run_bass_kernel_spmd(
        _get_nc(), _shard_inputs(left, right), list(range(NCORES)), **run_kwargs
    )
    return _gather_outputs(res.results), res


def kernel(**inputs):
    left = np.asarray(inputs["left_feature"], dtype=np.float32)
    right = np.asarray(inputs["right_feature"], dtype=np.float32)
    max_disp = int(np.asarray(inputs["max_disp"]))
    assert left.shape == (B, C, H, W), left.shape
    assert right.shape == (B, C, H, W), right.shape
    assert max_disp // 4 == D, max_disp
    out, _ = run_sharded(left, right)
    return out

